# revision 1
# baseline (speedup 1.0000x reference)
"""Fused multi-head self-attention (concat-head, scale=sqrt(d_model)) on 8 trn2 cores.

Sharding: batch(4) x key-half(2) -> 8 cores. Each core:
  - input xT = x[b].T [F=512, T=2048], pre-transposed AND column-rotated on
    host so that this core's key-half is always columns 0:1024 (the rotation
    keeps the device program identical across cores; host un-rolls outputs).
  - host fuses M = Wq @ Wk^T (fp64, exact), so scores = Xq M Xkv^T needs a
    single projection yT = M^T @ xqT instead of separate q/k projections
  - computes yT (all queries), v = xkvT^T @ Wv (its 1024 keys only)
  - scoresT[s, tq] = xkvT_slice.T @ yT (contract feature dim), then
    expT = exp(scoresT / sqrt(512)) (no max-subtraction needed: scores O(1))
  - partial out[tq, p] = expT.T @ v and partial row-sums (ones-vector matmul),
    both returned unnormalized; host combines the two key-halves:
    out = (o0 + o1) / (s0 + s1).
All matmul operands are bf16 (fp32 accumulate); exp/sums in fp32.
"""

import os
from contextlib import ExitStack

import numpy as np
import ml_dtypes

import concourse.bass as bass
import concourse.tile as tile
import concourse.mybir as mybir
from concourse import bacc
from concourse.bass_utils import run_bass_kernel_spmd

B, T, F, P = 4, 2048, 512, 512
NCORES = 8
KSPLIT = NCORES // B          # key-dim split per batch
TKV = T // KSPLIT             # 1024 keys per core
SCALE = 1.0 / float(np.sqrt(512.0))

FT = F // 128     # 4 f-tiles (contraction of projections)
PT = P // 128     # 4 p-tiles (contraction of scores)
ST = TKV // 128   # 8 s-tiles (keys per core)
NCH = T // 512    # 4 query chunks of 512
F32 = mybir.dt.float32

# matmul dtype: "bf16" (1 cyc/row) | "fp32" (4 cyc/row, exact)
KDT = os.environ.get("KDT", "bf16")


def _mm_dtypes():
    if KDT == "bf16":
        return mybir.dt.bfloat16, np.dtype(ml_dtypes.bfloat16)
    elif KDT == "fp32":
        return mybir.dt.float32, np.dtype(np.float32)
    else:
        raise ValueError(KDT)


def _attn_body(ctx, tc, xqt, wm, wv, out, sums):
    nc = tc.nc
    DT, _ = _mm_dtypes()
    Exp = mybir.ActivationFunctionType.Exp

    consts = ctx.enter_context(tc.tile_pool(name="consts", bufs=1))
    persist = ctx.enter_context(tc.tile_pool(name="persist", bufs=1))
    exp_pool = ctx.enter_context(tc.tile_pool(name="expp", bufs=2))
    out_pool = ctx.enter_context(tc.tile_pool(name="outsb", bufs=3))
    small = ctx.enter_context(tc.tile_pool(name="small", bufs=2))
    ps_sc = ctx.enter_context(tc.tile_pool(name="pssc", bufs=3, space="PSUM"))
    ps_out = ctx.enter_context(tc.tile_pool(name="psout", bufs=4, space="PSUM"))
    ps_sum = ctx.enter_context(tc.tile_pool(name="pssum", bufs=1, space="PSUM"))

    # ---- PE warmup: junk matmuls with no DMA deps, overlap the HAM ramp
    # and the initial input DMAs ----
    junk = consts.tile([128, 128], DT, tag="junk", name="junk")
    nc.gpsimd.memset(junk, 0.0)
    for w in range(40):
        wu = ps_sc.tile([128, 128], F32, tag="sc", name="wu")
        nc.tensor.matmul(wu, junk, junk, start=True, stop=True)

    # ---- load weights + inputs (wq/xq chunk 0 first so qT starts early) ----
    wm_sb = [consts.tile([128, P], DT, tag=f"wm{i}", name=f"wm{i}") for i in range(FT)]
    wv_sb = [consts.tile([128, P], DT, tag=f"wv{i}", name=f"wv{i}") for i in range(FT)]
    xq_sb = [consts.tile([128, T], DT, tag=f"xq{i}", name=f"xq{i}") for i in range(FT)]
    # v/kT inputs first (smallest working set), need-ordered round-robin on
    # all three DMA queues; qT inputs follow (needed only ~24us in)
    qdma = [nc.sync, nc.gpsimd, nc.scalar]
    di = 0

    def dma_in(out_ap, in_ap):
        nonlocal di
        qdma[di % 3].dma_start(out=out_ap, in_=in_ap)
        di += 1

    for i in range(FT):
        dma_in(wv_sb[i], wv[i * 128 : (i + 1) * 128, :])
        dma_in(xq_sb[i][:, 0:512], xqt[i * 128 : (i + 1) * 128, 0:512])
    for i in range(FT):
        dma_in(xq_sb[i][:, 512:1024], xqt[i * 128 : (i + 1) * 128, 512:1024])
    for i in range(FT):
        dma_in(wm_sb[i], wm[i * 128 : (i + 1) * 128, :])
    for c in range(TKV // 512, T // 512):
        for i in range(FT):
            dma_in(
                xq_sb[i][:, c * 512 : (c + 1) * 512],
                xqt[i * 128 : (i + 1) * 128, c * 512 : (c + 1) * 512],
            )

    ones_sb = consts.tile([128, 1], DT, tag="ones", name="ones")
    nc.vector.memset(ones_sb, 1.0)

    # ---- projections, v first to match DMA arrival; yT chunks run inside
    # the attention loop where they have huge DMA slack ----
    yt_sb = [persist.tile([128, T], DT, tag=f"yt{m}", name=f"yt{m}") for m in range(PT)]
    v_sb = [persist.tile([128, P], DT, tag=f"v{s}", name=f"v{s}") for s in range(ST)]
    for s in range(ST):
        ps = ps_sc.tile([128, 512], F32, tag="sc", name="ps_v")
        for kf in range(FT):
            nc.tensor.matmul(
                ps,
                xq_sb[kf][:, s * 128 : (s + 1) * 128],
                wv_sb[kf],
                start=kf == 0,
                stop=kf == FT - 1,
            )
        nc.vector.tensor_copy(out=v_sb[s], in_=ps)
        wuf = ps_sc.tile([128, 128], F32, tag="sc", name="wuf")
        nc.tensor.matmul(wuf, junk, junk, start=True, stop=True)

    # ---- attention, per query chunk of 512; out-accumulation s-outer,
    # pipelined one s-step behind scores so PE never waits on ACT exp ----
    for c in range(NCH):
        qs = slice(c * 512, (c + 1) * 512)
        # yT for this query chunk only -- spreads the xq DMA need across the
        # whole kernel instead of front-loading it
        for m in range(PT):
            ps = ps_sc.tile([128, 512], F32, tag="sc", name="ps_y")
            for kf in range(FT):
                nc.tensor.matmul(
                    ps,
                    wm_sb[kf][:, m * 128 : (m + 1) * 128],
                    xq_sb[kf][:, qs],
                    start=kf == 0,
                    stop=kf == FT - 1,
                )
            nc.vector.tensor_copy(out=yt_sb[m][:, qs], in_=ps)
        if c < NCH - 1:
            exp_c = [
                exp_pool.tile([128, 512], DT, tag=f"exp{s % 3}", name=f"exp{s % 3}")
                for s in range(ST)
            ]
        else:
            exp_c = [
                exp_pool.tile(
                    [128, 512], DT, tag=f"expL{s}", name=f"expL{s}", bufs=1
                )
                for s in range(ST)
            ]
        # bf16 running-sum for the exp row-sums: rounding only touches small
        # partials (<=8), the big accumulation stays in the fp32 matmul; the
        # running form leaves only one add after the final exp
        t1 = [
            exp_pool.tile([128, 512], DT, tag=f"t1_{i % 2}", name=f"t1_{i % 2}")
            for i in range(4)
        ]
        run = [None] + [
            exp_pool.tile([128, 512], DT, tag=f"run{i % 2}", name=f"run{i % 2}")
            for i in range(1, 4)
        ]
        sums_ps = ps_sum.tile([1, 512], F32, tag="sums", name="sums_ps")
        po = [
            ps_out.tile([128, 512], F32, tag=f"out{t4}", name=f"po{t4}", bufs=1)
            for t4 in range(4)
        ]

        def scores_step(s):
            ps = ps_sc.tile([128, 512], F32, tag="sc", name="ps_sc")
            for pm in range(PT):
                nc.tensor.matmul(
                    ps,
                    xq_sb[pm][:, s * 128 : (s + 1) * 128],
                    yt_sb[pm][:, qs],
                    start=pm == 0,
                    stop=pm == PT - 1,
                )
            if c == NCH - 1 and s == ST - 1:
                # tail: emit exp in 4 column pieces so the final out-matmuls
                # (lhsT = one 128-col piece each) start as pieces land
                for piece in range(4):
                    cs = slice(piece * 128, (piece + 1) * 128)
                    nc.scalar.activation(
                        out=exp_c[s][:, cs], in_=ps[:, cs], func=Exp, scale=SCALE
                    )
            else:
                nc.scalar.activation(out=exp_c[s], in_=ps, func=Exp, scale=SCALE)

        def out_step(s):
            for t4 in range(4):
                nc.tensor.matmul(
                    po[t4],
                    exp_c[s][:, t4 * 128 : (t4 + 1) * 128],
                    v_sb[s],
                    start=s == 0,
                    stop=s == ST - 1,
                    skip_group_check=True,
                )
                if s == ST - 1:
                    # copy out as soon as this tile's accumulation is done, so
                    # the PSUM bank frees up for the next chunk quickly; halves
                    # on DVE+ACT in parallel to cut release latency
                    tt = c * 4 + t4
                    osb = out_pool.tile([128, 512], F32, tag="osb", name="osb")
                    nc.vector.tensor_copy(out=osb[:, 0:256], in_=po[t4][:, 0:256])
                    nc.scalar.copy(out=osb[:, 256:512], in_=po[t4][:, 256:512])
                    [nc.sync, nc.scalar, nc.gpsimd, nc.sync][t4].dma_start(
                        out=out[tt * 128 : (tt + 1) * 128, :], in_=osb
                    )
            last = c == NCH - 1
            if s % 2 == 1:
                nc.vector.tensor_add(t1[s // 2], exp_c[s - 1], exp_c[s])
                if s > 1 and not (last and s == ST - 1):
                    nc.vector.tensor_add(
                        run[s // 2], run[s // 2 - 1] if s > 3 else t1[0], t1[s // 2]
                    )
            # row-sums of exp over the partition dim via ones-matmul; on the
            # last chunk split it so only one DVE add remains after exp(ST-1)
            if last:
                if s == ST - 3:
                    nc.tensor.matmul(
                        sums_ps, ones_sb, run[ST // 2 - 2], start=True, stop=False,
                        skip_group_check=True,
                    )
                elif s == ST - 1:
                    nc.tensor.matmul(
                        sums_ps, ones_sb, t1[ST // 2 - 1], start=False, stop=True,
                        skip_group_check=True,
                    )
            elif s == ST - 1:
                nc.tensor.matmul(
                    sums_ps, ones_sb, run[ST // 2 - 1], start=True, stop=True,
                    skip_group_check=True,
                )

        def last_step(s):
            nc.tensor.matmul(
                po[0],
                exp_c[s][:, 0:128],
                v_sb[s],
                start=s == 0,
                stop=s == ST - 1,
                skip_group_check=True,
            )
            if s % 2 == 1:
                nc.vector.tensor_add(t1[s // 2], exp_c[s - 1], exp_c[s])
                if s > 1 and s != ST - 1:
                    nc.vector.tensor_add(
                        run[s // 2], run[s // 2 - 1] if s > 3 else t1[0], t1[s // 2]
                    )
            if s == ST - 3:
                nc.tensor.matmul(
                    sums_ps, ones_sb, run[ST // 2 - 2], start=True, stop=False,
                    skip_group_check=True,
                )
            elif s == ST - 1:
                nc.tensor.matmul(
                    sums_ps, ones_sb, t1[ST // 2 - 1], start=False, stop=True,
                    skip_group_check=True,
                )
                tt = c * 4
                osb = out_pool.tile([128, 512], F32, tag="osb", name="osb")
                nc.vector.tensor_copy(out=osb[:, 0:256], in_=po[0][:, 0:256])
                nc.scalar.copy(out=osb[:, 256:512], in_=po[0][:, 256:512])
                nc.sync.dma_start(out=out[tt * 128 : (tt + 1) * 128, :], in_=osb)

        if c < NCH - 1:
            scores_step(0)
            for s in range(1, ST):
                scores_step(s)
                out_step(s - 1)
            out_step(ST - 1)
        else:
            # staggered tail: only po[0] accumulates during the scores
            # pipeline; po[1..3] then stream dense with copy+DMA after each,
            # so output transfers overlap the remaining matmuls
            scores_step(0)
            for s in range(1, ST):
                scores_step(s)
                last_step(s - 1)
            last_step(ST - 1)
            for t4 in range(1, 4):
                tt = c * 4 + t4
                for s in range(ST):
                    nc.tensor.matmul(
                        po[t4],
                        exp_c[s][:, t4 * 128 : (t4 + 1) * 128],
                        v_sb[s],
                        start=s == 0,
                        stop=s == ST - 1,
                        skip_group_check=True,
                    )
                osb = out_pool.tile([128, 512], F32, tag="osb", name="osb")
                nc.vector.tensor_copy(out=osb[:, 0:256], in_=po[t4][:, 0:256])
                nc.scalar.copy(out=osb[:, 256:512], in_=po[t4][:, 256:512])
                [nc.sync, nc.scalar, nc.gpsimd, nc.sync][t4].dma_start(
                    out=out[tt * 128 : (tt + 1) * 128, :], in_=osb
                )

        sums_sb = small.tile([1, 512], F32, tag="sums_sb", name="sums_sb")
        nc.vector.tensor_copy(out=sums_sb, in_=sums_ps)
        nc.sync.dma_start(out=sums[0:1, qs], in_=sums_sb)


_CACHE = {}


def _get_compiled():
    key = KDT
    if key in _CACHE:
        return _CACHE[key]
    DT, _ = _mm_dtypes()
    nc = bacc.Bacc(
        "TRN2",
        target_bir_lowering=False,
        debug=False,
        enable_asserts=False,
        num_devices=NCORES,
        num_swdge_queues=2,
    )
    xqt = nc.dram_tensor("xqt", [F, T], DT, kind="ExternalInput").ap()
    wm = nc.dram_tensor("wm", [F, P], DT, kind="ExternalInput").ap()
    wv = nc.dram_tensor("wv", [F, P], DT, kind="ExternalInput").ap()
    out = nc.dram_tensor("out", [T, P], F32, kind="ExternalOutput").ap()
    sums = nc.dram_tensor("sums", [1, T], F32, kind="ExternalOutput").ap()
    with tile.TileContext(nc) as tc, ExitStack() as ctx:
        _attn_body(ctx, tc, xqt, wm, wv, out, sums)
    nc.compile()
    _CACHE[key] = nc
    return nc


def kernel(x, Wq, Wk, Wv, _trace=False):
    _, np_dt = _mm_dtypes()
    nc = _get_compiled()
    # fused scores weight: scores = Xq (Wq Wk^T) Xkv^T; fp64 on host, exact
    wm_c = np.ascontiguousarray(
        (Wq.astype(np.float64) @ Wk.astype(np.float64).T).astype(np_dt)
    )
    wv_c = np.ascontiguousarray(Wv.astype(np_dt))
    xT = [np.ascontiguousarray(x[b].T.astype(np_dt)) for b in range(B)]
    in_maps = []
    for core in range(NCORES):
        b, h = divmod(core, KSPLIT)
        in_maps.append(
            {
                # rotate so this core's key-half sits in columns 0:TKV
                "xqt": np.ascontiguousarray(np.roll(xT[b], -h * TKV, axis=1)),
                "wm": wm_c,
                "wv": wv_c,
            }
        )
    res = run_bass_kernel_spmd(
        nc, in_maps, core_ids=list(range(NCORES)), trace=_trace
    )
    out = np.empty((B, T, P), np.float32)
    for b in range(B):
        o = np.zeros((T, P), np.float32)
        s = np.zeros(T, np.float32)
        for h in range(KSPLIT):
            r = res.results[b * KSPLIT + h]
            # un-rotate the query axis (device query j = original (j + h*TKV) % T)
            o += np.roll(r["out"], h * TKV, axis=0)
            s += np.roll(r["sums"][0], h * TKV)
        out[b] = o / s[:, None]
    if _trace:
        return out, res
    return out



# revision 2
# speedup vs baseline: 1.2544x; 1.2544x over previous
"""Fused multi-head self-attention (concat-head, scale=sqrt(d_model)) on 8 trn2 cores.

Sharding: batch(4) x key-half(2) -> 8 cores, host-rotated so every core runs an
identical program with its key-half in columns 0:1024 (host un-rolls outputs).

Math per core (keys S=1024 local, queries T=2048 all):
  scores = Xq M Xkv^T with M = Wq Wk^T fused on host (fp64).  Re-associated as
  z = M Xkv^T (only local keys -> half the projection work of y = Xq M), then
  scoresT = z^T Xq^T.
  a = exp(scores/sqrt(512)); out_num = sum_s a_s v_s = colsum(V) + delta V with
  delta = a - 1.  colsum(V) = (sum_s x_s) Wv is computed EXACTLY on host (fp64),
  the device only computes delta V.  This keeps fp8 quantization error on the
  small delta (|delta| ~ 0.2) instead of on a (~1.0), and off the rank-1 term.

fp8 (e4m3) DoubleRow matmuls (2 k-tiles per instruction, ~1.44x over bf16) for
the two big T x T matmuls: scoresT (z8 fp8 x xq8 fp8) and delta V (d8 fp8 x v8
fp8). z/v projections stay bf16 (v's error is not damped -> needs bf16; z is
cheap). Device returns unnormalized delta-V partials (bf16) + row sums of delta
(fp32); host adds colsum + 2048 and divides across the two key-halves.
"""

import os
from contextlib import ExitStack

import numpy as np
import ml_dtypes

import concourse.bass as bass
import concourse.tile as tile
import concourse.mybir as mybir
from concourse import bacc
from concourse.bass_utils import run_bass_kernel_spmd

B, T, F, P = 4, 2048, 512, 512
NCORES = 8
KSPLIT = NCORES // B          # key-dim split per batch
TKV = T // KSPLIT             # 1024 keys per core
SCALE = 1.0 / float(np.sqrt(512.0))

FT = F // 128     # 4 j-tiles (contraction of projections)
PT = P // 128     # 4 i-tiles (contraction of scores)
ST = TKV // 128   # 8 s-tiles (keys per core)
SP = ST // 2      # 4 s-pairs (DoubleRow granule)
NCH = T // 512    # 4 query chunks of 512
F32 = mybir.dt.float32
BF = mybir.dt.bfloat16
E4 = mybir.dt.float8e4
DR = mybir.MatmulPerfMode.DoubleRow

NP_BF = np.dtype(ml_dtypes.bfloat16)
NP_E4 = np.dtype(ml_dtypes.float8_e4m3)   # IEEE e4m3 == TRN FP8_EXP4 (max 240)

WARMUP = int(os.environ.get("WARMUP", "40"))


def _attn_body(ctx, tc, xq8, xkv, wmt, wv, out, sums):
    nc = tc.nc
    Exp = mybir.ActivationFunctionType.Exp

    consts = ctx.enter_context(tc.tile_pool(name="consts", bufs=1))
    persist = ctx.enter_context(tc.tile_pool(name="persist", bufs=1))
    dpool = ctx.enter_context(tc.tile_pool(name="dpool", bufs=2))
    ebpool = ctx.enter_context(tc.tile_pool(name="ebp", bufs=3))
    out_pool = ctx.enter_context(tc.tile_pool(name="outsb", bufs=3))
    small = ctx.enter_context(tc.tile_pool(name="small", bufs=2))
    ps_sc = ctx.enter_context(tc.tile_pool(name="pssc", bufs=3, space="PSUM"))
    ps_out = ctx.enter_context(tc.tile_pool(name="psout", bufs=4, space="PSUM"))
    ps_sum = ctx.enter_context(tc.tile_pool(name="pssum", bufs=1, space="PSUM"))

    # ---- PE warmup: junk matmuls with no DMA deps, overlap the HAM ramp
    # and the initial input DMAs ----
    junk = consts.tile([128, 128], BF, tag="junk", name="junk")
    nc.gpsimd.memset(junk, 0.0)
    for w in range(WARMUP):
        wu = ps_sc.tile([128, 128], F32, tag="sc", name="wu")
        nc.tensor.matmul(wu, junk, junk, start=True, stop=True)

    # ---- load inputs; v path (wv/xkv) first, then wmt for z, then xq8 ----
    wv_sb = [consts.tile([128, P], BF, tag=f"wv{j}", name=f"wv{j}") for j in range(FT)]
    wmt_sb = [consts.tile([128, P], BF, tag=f"wm{j}", name=f"wm{j}") for j in range(FT)]
    xkv_sb = consts.tile([128, FT, TKV], BF, tag="xkv", name="xkv")
    xq8_sb = consts.tile([128, PT, T], E4, tag="xq8", name="xq8")

    qdma = [nc.sync, nc.gpsimd, nc.scalar]
    di = 0

    def dma_in(out_ap, in_ap):
        nonlocal di
        qdma[di % 3].dma_start(out=out_ap, in_=in_ap)
        di += 1

    for j in range(FT):
        dma_in(wv_sb[j], wv[j * 128 : (j + 1) * 128, :])
        dma_in(xkv_sb[:, j, :], xkv[:, j, :])
    for j in range(FT):
        dma_in(wmt_sb[j], wmt[j * 128 : (j + 1) * 128, :])
    for c in range(NCH):
        dma_in(
            xq8_sb[:, :, c * 512 : (c + 1) * 512],
            xq8[:, :, c * 512 : (c + 1) * 512],
        )

    ones_sb = consts.tile([128, 1], BF, tag="ones", name="ones")
    nc.vector.memset(ones_sb, 1.0)

    # ---- v projection (bf16, fp8 result for the DR delta-V matmul) ----
    v8 = [persist.tile([128, 2, P], E4, tag=f"v8_{k}", name=f"v8_{k}") for k in range(SP)]
    for s in range(ST):
        ps = ps_sc.tile([128, 512], F32, tag="sc", name="ps_v")
        for j in range(FT):
            nc.tensor.matmul(
                ps,
                xkv_sb[:, j, s * 128 : (s + 1) * 128],
                wv_sb[j],
                start=j == 0,
                stop=j == FT - 1,
            )
        nc.vector.tensor_copy(out=v8[s // 2][:, s % 2, :], in_=ps)
        wuf = ps_sc.tile([128, 128], F32, tag="sc", name="wuf")
        nc.tensor.matmul(wuf, junk, junk, start=True, stop=True)

    # ---- z = M Xkv^T (bf16 compute, fp8 result), local keys only ----
    z8 = persist.tile([128, PT, TKV], E4, tag="z8", name="z8")
    for sc in range(TKV // 512):
        for i in range(PT):
            ps = ps_sc.tile([128, 512], F32, tag="sc", name="ps_z")
            for j in range(FT):
                nc.tensor.matmul(
                    ps,
                    wmt_sb[j][:, i * 128 : (i + 1) * 128],
                    xkv_sb[:, j, sc * 512 : (sc + 1) * 512],
                    start=j == 0,
                    stop=j == FT - 1,
                )
            nc.vector.tensor_copy(out=z8[:, i, sc * 512 : (sc + 1) * 512], in_=ps)

    # ---- attention: per query chunk of 512; DoubleRow fp8 matmuls ----
    for c in range(NCH):
        qs = slice(c * 512, (c + 1) * 512)
        d8 = [
            dpool.tile([128, 2, 512], E4, tag=f"d8_{k % 2}", name=f"d8_{k % 2}")
            for k in range(SP)
        ]
        tk = [
            dpool.tile([128, 512], BF, tag=f"tk_{k % 2}", name=f"tk_{k % 2}")
            for k in range(SP)
        ]
        u01 = dpool.tile([128, 512], BF, tag="u01", name="u01")
        u23 = dpool.tile([128, 512], BF, tag="u23", name="u23")
        run = dpool.tile([128, 512], BF, tag="run", name="run")
        po = [
            ps_out.tile([128, 512], F32, tag=f"out{t4}", name=f"po{t4}", bufs=1)
            for t4 in range(4)
        ]

        def scores_step(s):
            ps = ps_sc.tile([128, 512], F32, tag="sc", name="ps_sc")
            for pr in range(2):
                nc.tensor.matmul(
                    ps,
                    z8[:, 2 * pr : 2 * pr + 2, s * 128 : (s + 1) * 128],
                    xq8_sb[:, 2 * pr : 2 * pr + 2, qs],
                    start=pr == 0,
                    stop=pr == 1,
                    perf_mode=DR,
                )
            eb = ebpool.tile([128, 512], BF, tag=f"eb{s % 3}", name=f"eb{s % 3}")
            nc.scalar.activation(out=eb, in_=ps, func=Exp, scale=SCALE)
            # delta = exp - 1, quantized to fp8 (error ~2.5% of 0.2, not of 1.0)
            nc.vector.tensor_scalar_sub(out=d8[s // 2][:, s % 2, :], in0=eb, scalar1=1.0)
            # running sums of delta on gpsimd (off the DVE critical path)
            k = s // 2
            if s % 2 == 1:
                nc.gpsimd.tensor_add(tk[k], d8[k][:, 0, :], d8[k][:, 1, :])
                if k == 1:
                    nc.gpsimd.tensor_add(u01, tk[0], tk[1])
                elif k == 3:
                    nc.gpsimd.tensor_add(u23, tk[2], tk[3])
                    nc.gpsimd.tensor_add(run, u01, u23)

        def out_step(k):
            for t4 in range(4):
                nc.tensor.matmul(
                    po[t4],
                    d8[k][:, :, t4 * 128 : (t4 + 1) * 128],
                    v8[k],
                    start=k == 0,
                    stop=k == SP - 1,
                    perf_mode=DR,
                    skip_group_check=True,
                )
                if k == SP - 1:
                    tt = c * 4 + t4
                    osb = out_pool.tile([128, 512], BF, tag="osb", name="osb")
                    nc.vector.tensor_copy(out=osb[:, 0:256], in_=po[t4][:, 0:256])
                    nc.scalar.copy(out=osb[:, 256:512], in_=po[t4][:, 256:512])
                    [nc.sync, nc.scalar, nc.gpsimd, nc.sync][t4].dma_start(
                        out=out[tt * 128 : (tt + 1) * 128, :], in_=osb
                    )

        # scores pipelined two steps ahead of the pair-granular out steps so
        # the PE never waits on ACT exp + DVE sub latency
        scores_step(0)
        scores_step(1)
        scores_step(2)
        scores_step(3)
        out_step(0)
        scores_step(4)
        scores_step(5)
        out_step(1)
        scores_step(6)
        scores_step(7)
        out_step(2)
        out_step(3)

        sums_ps = ps_sum.tile([1, 512], F32, tag="sums", name="sums_ps")
        nc.tensor.matmul(
            sums_ps, ones_sb, run, start=True, stop=True, skip_group_check=True
        )
        sums_sb = small.tile([1, 512], F32, tag="sums_sb", name="sums_sb")
        nc.vector.tensor_copy(out=sums_sb, in_=sums_ps)
        nc.sync.dma_start(out=sums[0:1, qs], in_=sums_sb)


_CACHE = {}


def _get_compiled():
    key = "fp8dr"
    if key in _CACHE:
        return _CACHE[key]
    nc = bacc.Bacc(
        "TRN2",
        target_bir_lowering=False,
        debug=False,
        enable_asserts=False,
        num_devices=NCORES,
        num_swdge_queues=2,
    )
    xq8 = nc.dram_tensor("xq8", [128, PT, T], E4, kind="ExternalInput").ap()
    xkv = nc.dram_tensor("xkv", [128, FT, TKV], BF, kind="ExternalInput").ap()
    wmt = nc.dram_tensor("wmt", [F, P], BF, kind="ExternalInput").ap()
    wv = nc.dram_tensor("wv", [F, P], BF, kind="ExternalInput").ap()
    out = nc.dram_tensor("out", [T, P], BF, kind="ExternalOutput").ap()
    sums = nc.dram_tensor("sums", [1, T], F32, kind="ExternalOutput").ap()
    with tile.TileContext(nc) as tc, ExitStack() as ctx:
        _attn_body(ctx, tc, xq8, xkv, wmt, wv, out, sums)
    nc.compile()
    _CACHE[key] = nc
    return nc


def kernel(x, Wq, Wk, Wv, _trace=False):
    nc = _get_compiled()
    # fused scores weight, transposed: wmt = (Wq Wk^T)^T = Wk Wq^T; fp64 exact
    wmt_c = np.ascontiguousarray(
        (Wk.astype(np.float64) @ Wq.astype(np.float64).T).astype(NP_BF)
    )
    wv_c = np.ascontiguousarray(Wv.astype(NP_BF))
    in_maps = []
    cs = []
    for b in range(B):
        # exact rank-1 term: colsum(V) = (sum_t x[b,t,:]) @ Wv, fp64
        cs.append(
            (x[b].astype(np.float64).sum(axis=0) @ Wv.astype(np.float64)).astype(
                np.float32
            )
        )
        xT = x[b].T  # [F, T]
        for h in range(KSPLIT):
            xr = np.roll(xT, -h * TKV, axis=1)
            xq8_h = np.ascontiguousarray(
                xr.reshape(PT, 128, T).transpose(1, 0, 2).astype(NP_E4)
            )
            xkv_h = np.ascontiguousarray(
                xr[:, 0:TKV].reshape(FT, 128, TKV).transpose(1, 0, 2).astype(NP_BF)
            )
            in_maps.append({"xq8": xq8_h, "xkv": xkv_h, "wmt": wmt_c, "wv": wv_c})
    res = run_bass_kernel_spmd(
        nc, in_maps, core_ids=list(range(NCORES)), trace=_trace
    )
    outp = np.empty((B, T, P), np.float32)
    for b in range(B):
        o = np.broadcast_to(cs[b][None, :], (T, P)).astype(np.float32).copy()
        s = np.full(T, float(T), np.float32)
        for h in range(KSPLIT):
            r = res.results[b * KSPLIT + h]
            # un-rotate the query axis (device query j = original (j + h*TKV) % T)
            o += np.roll(np.asarray(r["out"]).astype(np.float32), h * TKV, axis=0)
            s += np.roll(np.asarray(r["sums"][0]).astype(np.float32), h * TKV)
        outp[b] = o / s[:, None]
    if _trace:
        return outp, res
    return outp
